# revision 1
# baseline (speedup 1.0000x reference)
"""Trainium2 Bass kernel for NGCF-style embedding propagation (8 NeuronCores).

Math (reference, with A = adj / (sqrt(row_sum*col_sum)+eps)):
  updated_user = LReLU(A.T @ (item@W1) + (item * (A.T @ user)) @ W2 + user)
  updated_item = LReLU(A   @ (user@W1) + (user * (A   @ item)) @ W2 + item)

The normalization separates: A = diag(s_r) @ adj @ diag(s_c),
s_r = 1/sqrt(row_sum), s_c = 1/sqrt(col_sum) (eps negligible: sums ~4096).
With Xr = s_r*[item@W1, user] (own rows) and Xc = s_c*[user@W1, item]:
  P_pre = adj.T @ Xr   (user side; s_c folded in at the very end since it is
                        row-constant and passes through the W2 term)
  Q     = adj   @ Xc   (item side; s_r applied at drain)

Sharding: row-shard adj across 8 cores (1024 rows each). Each core:
 - streams its fp32 slice from HBM ONCE in 1 MiB chunks, caching it in SBUF
   as fp16 (16 MiB/core), fusing the cast with row-sum accumulation,
   and computing column partial sums with PE matmuls against ones,
 - AllReduce (32 KiB) for global column sums,
 - P: stationary = Xr[ub] (128x128), moving = natural-layout cache, N=512
   matmuls accumulating over ub in PSUM -> P^T blocks [d, a]; starts right
   after phase A (independent of the AllReduce) so the ReduceScatter of the
   2 MiB bf16 partials overlaps the Q-direction work,
 - Q: SBUF->SBUF DMA-xbar transposes of the cache ([128,1024] per instr,
   split across both HWDGE rings), stationary = Xc[cb], moving = transposed
   chunks, N=512, accumulating over cb in PSUM -> Q^T [d, u],
 - finishes both output slices locally (PE 128x128 transposes back to [row,d],
   W2 terms, residual adds, leaky-relu as max(x, 0.2x) on DVE).
"""

import numpy as np

N = 8192
D = 64
NCORES = 8
U = N // NCORES          # rows per core = 1024
UB = U // 128            # 128-row blocks per core = 8
CB = N // 128            # 128-col blocks = 64
LDF = 4096               # fp32 load-chunk free size
NLD = N // LDF           # load chunks per 128-row block = 2
CAST = LDF // 2          # each half cast on a different engine
SUP = 8                  # col-blocks per transpose superblock
NSUP = CB // SUP         # 8
ASUP = 16                # a-superblocks for P (512 cols each)

_CACHE = {}


def _build(dbg=False, single=False):
    import concourse.bass as bass
    import concourse.bacc as bacc
    import concourse.mybir as mybir
    import concourse.tile as tile
    from concourse import masks

    f32 = mybir.dt.float32
    f16 = mybir.dt.float16
    bf16 = mybir.dt.bfloat16
    AF = mybir.ActivationFunctionType
    ALU = mybir.AluOpType
    ds = bass.ds

    nc = bacc.Bacc("TRN2", target_bir_lowering=False, debug=False,
                   num_devices=(1 if single else NCORES), enable_asserts=False)

    adj = nc.dram_tensor("adj", [U, N], f32, kind="ExternalInput").ap()
    user_full = nc.dram_tensor("user_full", [N, D], f32, kind="ExternalInput").ap()
    item_full = nc.dram_tensor("item_full", [N, D], f32, kind="ExternalInput").ap()
    user_own = nc.dram_tensor("user_own", [U, D], f32, kind="ExternalInput").ap()
    item_own = nc.dram_tensor("item_own", [U, D], f32, kind="ExternalInput").ap()
    w1 = nc.dram_tensor("w1", [D, D], f32, kind="ExternalInput").ap()
    w2 = nc.dram_tensor("w2", [D, D], f32, kind="ExternalInput").ap()
    upd_user = nc.dram_tensor("upd_user", [U, D], f32, kind="ExternalOutput").ap()
    upd_item = nc.dram_tensor("upd_item", [U, D], f32, kind="ExternalOutput").ap()
    if dbg:
        d_rowsum = nc.dram_tensor("d_rowsum", [128, UB], f32, kind="ExternalOutput").ap()
        d_colsum = nc.dram_tensor("d_colsum", [128, CB], f32, kind="ExternalOutput").ap()
        d_p = nc.dram_tensor("d_p", [CB, 128, 128], bf16, kind="ExternalOutput").ap()
        d_q = nc.dram_tensor("d_q", [128, UB * 128], f32, kind="ExternalOutput").ap()
        d_xc = nc.dram_tensor("d_xc", [128, CB, 2 * D], f16, kind="ExternalOutput").ap()
        d_adjt = nc.dram_tensor("d_adjt", [128, UB, SUP, 128], f16,
                                kind="ExternalOutput").ap()

    groups = [list(range(NCORES))]

    with tile.TileContext(nc) as tc:
        with (
            tc.tile_pool(name="persist", bufs=1) as persist,
            tc.tile_pool(name="big", bufs=2) as bigp,       # ld (A) + adjT (C)
            tc.tile_pool(name="embld", bufs=2) as embld,
            tc.tile_pool(name="small", bufs=2) as small,
            tc.tile_pool(name="xc", bufs=8) as xcp,
            tc.tile_pool(name="pstage", bufs=3) as pstagep,
            tc.tile_pool(name="dbgp", bufs=1) as dbgp,
            tc.tile_pool(name="psum_small", bufs=2, space="PSUM") as psum_small,
            tc.tile_pool(name="psum_p", bufs=3, space="PSUM") as psum_pp,
            tc.tile_pool(name="psum_big", bufs=1, space="PSUM") as psum_big,
            tc.tile_pool(name="dram", bufs=1, space="DRAM") as dram,
        ):
            # ---------------- persistent tiles
            cache = persist.tile([128, UB, N], f16)          # 128 KiB/part
            uown = persist.tile([128, UB, D], f32)           # 2
            iown = persist.tile([128, UB, D], f32)           # 2
            xr = persist.tile([128, UB, 2 * D], f16)         # 2
            rowsum_parts = persist.tile([128, UB, 2 * NLD], f32)
            rowsum = persist.tile([128, UB], f32)
            s_r = persist.tile([128, UB], f32)
            s_c = persist.tile([128, CB], f32)
            colsum = persist.tile([128, CB], f32)
            w1_hf = persist.tile([D, D], f16)
            w2_hf = persist.tile([D, D], f16)
            ones_hf = persist.tile([128, 1], f16)
            onerow = persist.tile([1, 128], f16)
            zrow = persist.tile([1, 512], f16)
            ident = persist.tile([128, 128], f16)
            ident32 = persist.tile([128, 128], f32)
            identbf = persist.tile([128, 128], bf16)
            qsb = persist.tile([128, UB, 128], bf16)         # 2 (Q^T staging)
            e1 = persist.tile([128, CB, D], f16)             # 8 (user@W1)
            item_hf = persist.tile([128, CB, D], f16)        # 8

            psum_col = psum_big.tile([128, CB], f32)         # 1 bank
            psum_qt = psum_big.tile([128, UB, 128], f32)     # 2 banks

            nc.gpsimd.memset(ones_hf[:], 1.0)
            nc.gpsimd.memset(onerow[:], 1.0)
            nc.gpsimd.memset(zrow[:], 0.0)
            masks.make_identity(nc, ident[:])
            masks.make_identity(nc, ident32[:])
            masks.make_identity(nc, identbf[:])

            # W1/W2 -> fp16
            wld = small.tile([D, 2 * D], f32, tag="wld")
            nc.scalar.dma_start(wld[:, 0:D], w1)
            nc.scalar.dma_start(wld[:, D:2 * D], w2)
            nc.vector.tensor_copy(w1_hf[:], wld[:, 0:D])
            nc.vector.tensor_copy(w2_hf[:], wld[:, D:2 * D])

            # own embeddings (fp32, small)
            uo_view = user_own.rearrange("(ub p) d -> p ub d", p=128)
            io_view = item_own.rearrange("(ub p) d -> p ub d", p=128)
            nc.scalar.dma_start(uown[:], uo_view)
            nc.scalar.dma_start(iown[:], io_view)

            # prime accumulator bank: start=True zero-matmul sets has_written
            # across the whole region; later matmuls accumulate-only.
            nc.tensor.matmul(psum_col[:], onerow[:], zrow[:, 0:CB],
                             start=True, stop=False, skip_group_check=True)

            # ---------------- phase A: stream adjacency slice
            adj_v = adj.rearrange("(ub p) n -> p ub n", p=128)
            for ub in range(UB):
                for ch in range(NLD):
                    ld = bigp.tile([128, LDF], f32, tag="big")
                    cs = slice(ch * LDF, (ch + 1) * LDF)
                    nc.sync.dma_start(ld[:], adj_v[:, ub, cs])
                    # fused cast->fp16 + row-sum, halves on ACT and DVE
                    h0 = slice(ch * LDF, ch * LDF + CAST)
                    h1 = slice(ch * LDF + CAST, (ch + 1) * LDF)
                    nc.scalar.activation(
                        cache[:, ub, h0], ld[:, 0:CAST], AF.Copy,
                        accum_out=rowsum_parts[:, ub, 2 * ch:2 * ch + 1])
                    nc.vector.tensor_scalar(
                        cache[:, ub, h1], ld[:, CAST:LDF], 1.0, 0.0, ALU.mult,
                        ALU.add,
                        accum_out=rowsum_parts[:, ub, 2 * ch + 1:2 * ch + 2])
                # finalize row sums -> s_r
                nc.vector.tensor_reduce(
                    rowsum[:, ub:ub + 1], rowsum_parts[:, ub], mybir.AxisListType.X,
                    ALU.add)
                sq = small.tile([128, 1], f32, tag="sq")
                nc.scalar.sqrt(sq[:], rowsum[:, ub:ub + 1])
                nc.vector.reciprocal(s_r[:, ub:ub + 1], sq[:])

                # column partial sums on PE (accumulate over ub; bank primed)
                for cb in range(CB):
                    nc.tensor.matmul(
                        psum_col[:, cb:cb + 1],
                        cache[:, ub, cb * 128:(cb + 1) * 128],
                        ones_hf[:],
                        start=False, stop=(ub == UB - 1),
                        skip_group_check=True)

                # Xr[ub] = s_r * [item_own@W1, user_own]
                ib = small.tile([128, D], f16, tag="ib")
                nc.vector.tensor_copy(ib[:], iown[:, ub])
                pt = psum_small.tile([D, 128], f16, tag="pe")
                nc.tensor.transpose(pt[:], ib[:], ident[:])
                ibt = small.tile([D, 128], f16, tag="ibt")
                nc.vector.tensor_copy(ibt[:], pt[:])
                pe = psum_small.tile([128, D], f32, tag="pe")
                nc.tensor.matmul(pe[:], ibt[:], w1_hf[:], start=True, stop=True)
                nc.scalar.activation(xr[:, ub, 0:D], pe[:], AF.Copy,
                                     scale=s_r[:, ub:ub + 1])
                nc.scalar.activation(xr[:, ub, D:2 * D], uown[:, ub], AF.Copy,
                                     scale=s_r[:, ub:ub + 1])

            # ---------------- prebuild e1 = user@W1 (no s_c needed; overlaps A)
            uf_view = user_full.rearrange("(b c p) d -> p b c d", p=128, c=SUP)
            for b in range(CB // SUP):
                ue = embld.tile([128, SUP, D], f32, tag="ue")
                nc.scalar.dma_start(ue[:], uf_view[:, b])
                for i in range(SUP):
                    cb = b * SUP + i
                    uhf = small.tile([128, D], f16, tag="uhf")
                    nc.vector.tensor_copy(uhf[:], ue[:, i])
                    ptu = psum_small.tile([D, 128], f16, tag="pe")
                    nc.tensor.transpose(ptu[:], uhf[:], ident[:])
                    ubt = small.tile([D, 128], f16, tag="ibt")
                    nc.vector.tensor_copy(ubt[:], ptu[:])
                    peu = psum_small.tile([128, D], f32, tag="pe")
                    nc.tensor.matmul(peu[:], ubt[:], w1_hf[:], start=True,
                                     stop=True)
                    nc.scalar.activation(e1[:, cb], peu[:], AF.Copy)

            # prefetch item embeddings -> fp16 resident (overlaps phase A)
            if_view2 = item_full.rearrange("(b c p) d -> p b c d", p=128, c=SUP)
            for b in range(CB // SUP):
                ie = embld.tile([128, SUP, D], f32, tag="ie")
                nc.scalar.dma_start(ie[:], if_view2[:, b])
                nc.any.tensor_copy(item_hf[:, b * SUP:(b + 1) * SUP], ie[:])

            # ---------------- column-sum AllReduce -> s_c (overlaps P below)
            col_sb = small.tile([128, CB], f32, tag="colsb")
            nc.vector.tensor_copy(col_sb[:], psum_col[:])
            col_in = dram.tile([128, CB], f32)
            col_out = dram.tile([128, CB], f32, addr_space="Shared")
            nc.scalar.dma_start(col_in[:], col_sb[:])
            if single:
                nc.scalar.dma_start(col_out[:], col_in[:])
            else:
                nc.gpsimd.collective_compute(
                    "AllReduce", mybir.AluOpType.add, replica_groups=groups,
                    ins=[col_in.opt()], outs=[col_out.opt()])
            nc.gpsimd.dma_start(colsum[:], col_out[:])
            sqc = small.tile([128, CB], f32, tag="sqc")
            nc.scalar.sqrt(sqc[:], colsum[:])
            nc.vector.reciprocal(s_c[:], sqc[:])

            # ---------------- P: stationary Xr[ub], moving natural cache
            # out = P^T blocks [128d, 512a]; only needs phase A results.
            p_in = dram.tile([CB, 128, 128], bf16)
            p_out = dram.tile([UB, 128, 128], bf16)

            def emit_p_asup(asup):
                pp = psum_pp.tile([128, 512], f32, tag="pp")
                for ub in range(UB):
                    nc.tensor.matmul(
                        pp[:], xr[:, ub], cache[:, ub, asup * 512:(asup + 1) * 512],
                        start=(ub == 0), stop=(ub == UB - 1),
                        skip_group_check=True)
                pst = pstagep.tile([128, 512], bf16, tag="pstage")
                nc.vector.tensor_copy(pst[:], pp[:])
                blk = slice(asup * 4, (asup + 1) * 4)
                nc.gpsimd.dma_start(p_in[blk].rearrange("b d c -> d b c"), pst[:])

            # ---------------- Q: transposed chunks, stationary Xc[cb]
            for sup in range(NSUP):
                # transpose superblock first: the xbar is the serial resource,
                # give its DMAs the head start; P matmuls below fill PE time
                adjt = bigp.tile([128, UB, SUP, 128], f16, tag="big")
                for ub in range(UB):
                    nc.sync.dma_start(
                        adjt[:, ub],
                        cache[:, ub, sup * SUP * 128:(sup + 1) * SUP * 128],
                        transpose=True)
                emit_p_asup(2 * sup)
                emit_p_asup(2 * sup + 1)
                if dbg and sup == 0:
                    nc.scalar.dma_start(d_adjt[:], adjt[:])

                for i in range(SUP):
                    cb = sup * SUP + i
                    # Xc[cb] = s_c[cb] * [e1[cb], item]  (fp16)
                    xc = xcp.tile([128, 2 * D], f16, tag="xc")
                    nc.scalar.activation(xc[:, 0:D], e1[:, cb], AF.Copy,
                                         scale=s_c[:, cb:cb + 1])
                    nc.scalar.activation(xc[:, D:2 * D], item_hf[:, cb], AF.Copy,
                                         scale=s_c[:, cb:cb + 1])
                    if dbg:
                        nc.scalar.dma_start(d_xc[:, cb], xc[:])

                    # Q^T accumulation: one group per PSUM bank -> start safe
                    nc.tensor.matmul(
                        psum_qt[:, 0:4], xc[:], adjt[:, 0:4, i],
                        start=(cb == 0), stop=(cb == CB - 1),
                        skip_group_check=True)
                    nc.tensor.matmul(
                        psum_qt[:, 4:8], xc[:], adjt[:, 4:8, i],
                        start=(cb == 0), stop=(cb == CB - 1),
                        skip_group_check=True)

            if single:
                nc.scalar.dma_start(p_out[:], p_in[0:UB])
            else:
                nc.gpsimd.collective_compute(
                    "ReduceScatter", mybir.AluOpType.add, replica_groups=groups,
                    ins=[p_in.opt()], outs=[p_out.opt()])

            if dbg:
                nc.scalar.dma_start(d_rowsum[:], rowsum[:])
                nc.scalar.dma_start(d_colsum[:], colsum[:])
                nc.scalar.dma_start(d_p[:], p_in[:])
                for qh in range(2):
                    q32 = dbgp.tile([128, UB // 2, 128], f32, tag="dbg_q")
                    nc.vector.tensor_copy(q32[:], psum_qt[:, qh * 4:(qh + 1) * 4])
                    nc.scalar.dma_start(d_q[:, qh * 512:(qh + 1) * 512], q32[:])

            # ---------------- finish item side (Q^T -> [u, d], scale, W2, add)
            uu_view = upd_user.rearrange("(ub p) d -> p ub d", p=128)
            ui_view = upd_item.rearrange("(ub p) d -> p ub d", p=128)

            nc.vector.tensor_copy(qsb[:], psum_qt[:])
            for ub in range(UB):
                qtt = psum_pp.tile([128, 128], bf16, tag="pp")
                nc.tensor.transpose(qtt[:], qsb[:, ub], identbf[:])
                g = small.tile([128, D], f16, tag="g")
                nc.vector.scalar_tensor_tensor(
                    g[:], qtt[:, D:2 * D], s_r[:, ub:ub + 1], uown[:, ub],
                    ALU.mult, ALU.mult)
                ptg = psum_small.tile([D, 128], f16, tag="pe")
                nc.tensor.transpose(ptg[:], g[:], ident[:])
                gt = small.tile([D, 128], f16, tag="ibt")
                nc.vector.tensor_copy(gt[:], ptg[:])
                pg = psum_small.tile([128, D], f32, tag="pe")
                nc.tensor.matmul(pg[:], gt[:], w2_hf[:], start=True, stop=True)
                t3 = small.tile([128, D], f32, tag="t3")
                nc.vector.scalar_tensor_tensor(
                    t3[:], qtt[:, 0:D], s_r[:, ub:ub + 1], iown[:, ub],
                    ALU.mult, ALU.add)
                s2 = small.tile([128, D], f32, tag="s2")
                nc.vector.tensor_add(s2[:], t3[:], pg[:])
                oi = small.tile([128, D], f32, tag="oi")
                nc.vector.scalar_tensor_tensor(
                    oi[:], s2[:], 0.2, s2[:], ALU.mult, ALU.max)
                nc.gpsimd.dma_start(ui_view[:, ub], oi[:])

            # ---------------- finish user side (post-RS)
            pid = nc.vector.partition_id()
            rsp = persist.tile([128, UB, 128], bf16)
            nc.gpsimd.dma_start(rsp[:], p_out[:].rearrange("b d c -> d b c"))
            for ub in range(UB):
                rtt = psum_pp.tile([128, 128], bf16, tag="pp")
                nc.tensor.transpose(rtt[:], rsp[:, ub], identbf[:])
                g2 = small.tile([128, D], f16, tag="g")
                nc.vector.tensor_mul(g2[:], rtt[:, D:2 * D], iown[:, ub])
                ptg2 = psum_small.tile([D, 128], f16, tag="pe")
                nc.tensor.transpose(ptg2[:], g2[:], ident[:])
                gt2 = small.tile([D, 128], f16, tag="ibt")
                nc.vector.tensor_copy(gt2[:], ptg2[:])
                pg2 = psum_small.tile([128, D], f32, tag="pe")
                nc.tensor.matmul(pg2[:], gt2[:], w2_hf[:], start=True, stop=True)
                # s2u = (rtt1 + pg2) * s_c[own cb] + user_own, distributed so
                # each DVE op reads at most one PSUM operand
                if single:
                    sc_ap = s_c[:, ub:ub + 1]
                else:
                    sc_ap = s_c[:, ds(pid * UB + ub, 1)]
                t1 = small.tile([128, D], f32, tag="t3")
                nc.vector.scalar_tensor_tensor(
                    t1[:], rtt[:, 0:D], sc_ap, uown[:, ub], ALU.mult, ALU.add)
                s2u = small.tile([128, D], f32, tag="s2")
                nc.vector.scalar_tensor_tensor(
                    s2u[:], pg2[:], sc_ap, t1[:], ALU.mult, ALU.add)
                ou = small.tile([128, D], f32, tag="oi")
                nc.vector.scalar_tensor_tensor(
                    ou[:], s2u[:], 0.2, s2u[:], ALU.mult, ALU.max)
                nc.gpsimd.dma_start(uu_view[:, ub], ou[:])

    nc.compile()
    return nc


def _get_nc(dbg=False):
    key = ("nc", dbg)
    if key not in _CACHE:
        _CACHE[key] = _build(dbg)
    return _CACHE[key]


def make_in_maps(user_embeddings, item_embeddings, adjacency_matrix, W1, W2):
    adj = np.ascontiguousarray(np.asarray(adjacency_matrix, dtype=np.float32))
    ue = np.ascontiguousarray(np.asarray(user_embeddings, dtype=np.float32))
    ie = np.ascontiguousarray(np.asarray(item_embeddings, dtype=np.float32))
    w1 = np.ascontiguousarray(np.asarray(W1, dtype=np.float32))
    w2 = np.ascontiguousarray(np.asarray(W2, dtype=np.float32))
    in_maps = []
    for k in range(NCORES):
        sl = slice(k * U, (k + 1) * U)
        in_maps.append({
            "adj": np.ascontiguousarray(adj[sl]),
            "user_full": ue,
            "item_full": ie,
            "user_own": np.ascontiguousarray(ue[sl]),
            "item_own": np.ascontiguousarray(ie[sl]),
            "w1": w1,
            "w2": w2,
        })
    return in_maps


def assemble(results):
    upd_user = np.concatenate([results[k]["upd_user"] for k in range(NCORES)], 0)
    upd_item = np.concatenate([results[k]["upd_item"] for k in range(NCORES)], 0)
    return upd_user, upd_item


def kernel(user_embeddings, item_embeddings, adjacency_matrix, W1, W2):
    import time
    import concourse.bass_utils as bass_utils
    nc = _get_nc()
    in_maps = make_in_maps(user_embeddings, item_embeddings, adjacency_matrix,
                           W1, W2)
    last = None
    for attempt in range(3):
        try:
            res = bass_utils.run_bass_kernel_spmd(
                nc, in_maps, core_ids=list(range(NCORES)), trace=False)
            return assemble(res.results)
        except Exception as e:  # transient NRT/axon failures
            last = e
            time.sleep(10)
    raise last



# revision 2
# speedup vs baseline: 1.2694x; 1.2694x over previous
"""Trainium2 Bass kernel for NGCF-style embedding propagation (8 NeuronCores).

Math (reference, with A = adj / (sqrt(row_sum*col_sum)+eps)):
  updated_user = LReLU(A.T @ (item@W1) + (item * (A.T @ user)) @ W2 + user)
  updated_item = LReLU(A   @ (user@W1) + (user * (A   @ item)) @ W2 + item)

Row-shard adj across 8 cores (1024 rows each). Per core, with
Xr = s_r*[item@W1, user] (own rows) and Xc = s_c*[user@W1, item] (all cols):
  P_pre = adj.T @ Xr  -> ReduceScatter over user blocks
  Q     = adj @ Xc    -> local (own rows)

Schedule (the point of this rewrite): loads are PANEL-major (8 panels of
1024 columns x 8 row-blocks). Per chunk the fp32 data is cast to a resident
fp16 natural cache (ACT, fused row-sum accumulation), PE-transposed into a
double-buffered per-panel adjT buffer (staged PSUM->SBUF on DVE), and column
partial sums accumulate on PE. After each panel its 32KiB column-sum slice is
AllReduced; s_c and Xc for that panel are built while the next panel loads,
and Q runs lagged one panel as natural [row, d] accumulation in PSUM. So the
transpose, colsums, Q, and e1=user@W1 all hide inside the 93us adjacency
load. Phase B is only: s_r, Xr, P (natural cache resident), ReduceScatter,
and the two finish loops.
"""

import numpy as np

N = 8192
D = 64
NCORES = 8
U = N // NCORES          # rows per core = 1024
UB = U // 128            # 128-row blocks per core = 8
CB = N // 128            # 128-col blocks = 64
PAN = 8                  # column panels
PCB = CB // PAN          # col blocks per panel = 8
PW = PCB * 128           # panel width = 1024

_CACHE = {}


def _build(dbg=False, single=False):
    import concourse.bass as bass
    import concourse.bacc as bacc
    import concourse.mybir as mybir
    import concourse.tile as tile
    from concourse import masks

    f32 = mybir.dt.float32
    f16 = mybir.dt.float16
    bf16 = mybir.dt.bfloat16
    AF = mybir.ActivationFunctionType
    ALU = mybir.AluOpType
    ds = bass.ds

    nc = bacc.Bacc("TRN2", target_bir_lowering=False, debug=False,
                   num_devices=(1 if single else NCORES), enable_asserts=False)

    adj = nc.dram_tensor("adj", [U, N], f32, kind="ExternalInput").ap()
    user_full = nc.dram_tensor("user_full", [N, D], f32, kind="ExternalInput").ap()
    item_full = nc.dram_tensor("item_full", [N, D], f32, kind="ExternalInput").ap()
    user_own = nc.dram_tensor("user_own", [U, D], f32, kind="ExternalInput").ap()
    item_own = nc.dram_tensor("item_own", [U, D], f32, kind="ExternalInput").ap()
    w1 = nc.dram_tensor("w1", [D, D], f32, kind="ExternalInput").ap()
    w2 = nc.dram_tensor("w2", [D, D], f32, kind="ExternalInput").ap()
    upd_user = nc.dram_tensor("upd_user", [U, D], f32, kind="ExternalOutput").ap()
    upd_item = nc.dram_tensor("upd_item", [U, D], f32, kind="ExternalOutput").ap()

    groups = [list(range(NCORES))]

    with tile.TileContext(nc) as tc:
        with (
            tc.tile_pool(name="persist", bufs=1) as persist,
            tc.tile_pool(name="ld", bufs=3) as ldp,
            tc.tile_pool(name="embld", bufs=1) as embld,
            tc.tile_pool(name="small", bufs=2) as small,
            tc.tile_pool(name="pstp", bufs=2) as pstp,
            tc.tile_pool(name="psum_small", bufs=2, space="PSUM") as psum_small,
            tc.tile_pool(name="ps2k", bufs=2, space="PSUM") as ps2k,
            tc.tile_pool(name="psum_big", bufs=1, space="PSUM") as psum_big,
            tc.tile_pool(name="dram", bufs=1, space="DRAM") as dram,
        ):
            # ---------------- persistent SBUF tiles
            cache = persist.tile([128, UB, N], f16)          # 128 KiB/part
            adjt = persist.tile([128, 2, PCB, UB, 128], f16)  # 32 KiB
            ei = persist.tile([128, CB, 2 * D], f16)         # 16 KiB (e1|item)
            xc = persist.tile([128, 2, PCB, 2 * D], f16)     # 4 KiB
            uown = persist.tile([128, UB, D], f16)           # 1
            iown = persist.tile([128, UB, D], f16)           # 1
            xr = persist.tile([128, UB, 2 * D], f16)         # 2 (x0r then Xr)
            rowsum_parts = persist.tile([128, UB, PAN], f32)
            s_r = persist.tile([128, UB], f32)
            s_c = persist.tile([128, CB], f32)
            out_stage = persist.tile([128, UB, D], f32)      # 2
            w1_hf = persist.tile([D, D], f16)
            w2_hf = persist.tile([D, D], f16)
            ones_hf = persist.tile([128, 1], f16)
            onerow = persist.tile([1, 128], f16)
            zrow = persist.tile([1, 128], f16)
            ident = persist.tile([128, 128], f16)
            identbf = persist.tile([128, 128], bf16)

            psum_q = psum_big.tile([128, UB, 128], f32)      # 2 banks
            psum_col = psum_big.tile([128, CB], f32)         # 1 bank

            nc.gpsimd.memset(ones_hf[:], 1.0)
            nc.gpsimd.memset(onerow[:], 1.0)
            nc.gpsimd.memset(zrow[:], 0.0)
            masks.make_identity(nc, ident[:])
            masks.make_identity(nc, identbf[:])

            # W1/W2 -> fp16
            wld = embld.tile([D, 2 * D], f32, tag="ue")
            nc.gpsimd.dma_start(wld[:, 0:D], w1)
            nc.gpsimd.dma_start(wld[:, D:2 * D], w2)
            nc.vector.tensor_copy(w1_hf[:], wld[:, 0:D])
            nc.vector.tensor_copy(w2_hf[:], wld[:, D:2 * D])

            # own embeddings (fp32, one DMA each)
            uo_view = user_own.rearrange("(ub p) d -> p ub d", p=128)
            io_view = item_own.rearrange("(ub p) d -> p ub d", p=128)
            nc.gpsimd.dma_start(uown[:], uo_view)
            nc.gpsimd.dma_start(iown[:], io_view)

            # prime accumulator banks (start=True zero-matmul sets
            # has_written across each region; later matmuls accumulate-only).
            nc.tensor.matmul(psum_col[:], onerow[:], zrow[:, 0:CB],
                             start=True, stop=False, skip_group_check=True)
            for qb in range(UB):
                nc.tensor.matmul(psum_q[:, qb], onerow[:], zrow[:],
                                 start=True, stop=False,
                                 skip_group_check=True)

            # x0r = [iown@W1, uown] (unscaled); s_r applied in phase B
            for ub in range(UB):
                ib = small.tile([128, D], f16, tag="ib")
                nc.vector.tensor_copy(ib[:], iown[:, ub])
                pt = psum_small.tile([D, 128], f16, tag="pe")
                nc.tensor.transpose(pt[:], ib[:], ident[:])
                ibt = small.tile([D, 128], f16, tag="ibt")
                nc.vector.tensor_copy(ibt[:], pt[:])
                pe = psum_small.tile([128, D], f32, tag="pe")
                nc.tensor.matmul(pe[:], ibt[:], w1_hf[:], start=True, stop=True)
                nc.scalar.activation(xr[:, ub, 0:D], pe[:], AF.Copy)
                nc.vector.tensor_copy(xr[:, ub, D:2 * D], uown[:, ub])

            # ei = [user@W1 | item] in natural c-order (overlaps phase A).
            # user side: per 128-block transpose -> matmul; batched in psum.
            uf_view = user_full.rearrange("(b c p) d -> p b c d", p=128, c=4)
            if_view = item_full.rearrange("(b c p) d -> p b c d", p=128, c=4)
            for b in range(CB // 4):
                ue = embld.tile([128, 4, D], f32, tag="ue")
                nc.gpsimd.dma_start(ue[:], uf_view[:, b])
                uhf = small.tile([128, 4, D], f16, tag="uhf")
                nc.vector.tensor_copy(uhf[:], ue[:])
                ptu = psum_small.tile([D, 4, 128], f16, tag="pe")
                for i in range(4):
                    nc.tensor.transpose(ptu[:, i], uhf[:, i], ident[:])
                ubt = small.tile([D, 4, 128], f16, tag="ubt")
                nc.vector.tensor_copy(ubt[:], ptu[:])
                peu = psum_small.tile([128, 4, D], f32, tag="pe")
                for i in range(4):
                    nc.tensor.matmul(peu[:, i], ubt[:, i], w1_hf[:],
                                     start=True, stop=True)
                nc.scalar.activation(ei[:, b * 4:(b + 1) * 4, 0:D], peu[:],
                                     AF.Copy)
            for b in range(CB // 4):
                ie = embld.tile([128, 4, D], f32, tag="ue")
                nc.gpsimd.dma_start(ie[:], if_view[:, b])
                nc.vector.tensor_copy(ei[:, b * 4:(b + 1) * 4, D:2 * D], ie[:])

            # per-panel column-sum AllReduce buffers
            col_in = []
            col_out = []
            for _pn in range(PAN):
                ci = dram.tile([128, PCB], f32, name=f"col_in{_pn}")
                co = dram.tile([128, PCB], f32, addr_space="Shared",
                               name=f"col_out{_pn}")
                col_in.append(ci)
                col_out.append(co)

            adj_v = adj.rearrange("(ub p) n -> p ub n", p=128)

            def emit_q(panel):
                """Q matmuls for a completed panel (lagged): natural [r, d]
                accumulation, one matmul per (cb, ub)."""
                buf = panel % 2
                for j in range(PCB):
                    cb = panel * PCB + j
                    for ub in range(UB):
                        nc.tensor.matmul(
                            psum_q[:, ub], adjt[:, buf, j, ub],
                            xc[:, buf, j],
                            start=False, stop=(cb == CB - 1 and ub == UB - 1),
                            skip_group_check=True)

            # ---------------- phase A: panel-major streaming
            for panel in range(PAN):
                cs = slice(panel * PW, (panel + 1) * PW)
                for ub in range(UB):
                    ld = ldp.tile([128, PW], f32, tag="ld")
                    nc.sync.dma_start(ld[:], adj_v[:, ub, cs])
                    # cast -> resident fp16 cache, fused row-sum accumulation
                    nc.scalar.activation(
                        cache[:, ub, cs], ld[:], AF.Copy,
                        accum_out=rowsum_parts[:, ub, panel:panel + 1])
                    # PE transposes -> PSUM (fp16), staged to adjT on DVE
                    pst = ps2k.tile([128, PCB, 128], f16, tag="s2k")
                    for j in range(PCB):
                        c0 = panel * PW + j * 128
                        nc.tensor.transpose(pst[:, j],
                                            cache[:, ub, c0:c0 + 128],
                                            ident[:])
                    nc.vector.tensor_copy(adjt[:, panel % 2, :, ub], pst[:])
                    # column partial sums (accumulate over ub)
                    for j in range(PCB):
                        cb = panel * PCB + j
                        c0 = cb * 128
                        nc.tensor.matmul(
                            psum_col[:, cb:cb + 1],
                            cache[:, ub, c0:c0 + 128], ones_hf[:],
                            start=False, stop=(panel == PAN - 1 and ub == UB - 1),
                            skip_group_check=True)
                    # lagged Q for previous panel: burst once AR is done
                    if panel > 0 and ub == 5:
                        emit_q(panel - 1)

                # panel column sums complete -> AllReduce -> s_c -> Xc
                csl = slice(panel * PCB, (panel + 1) * PCB)
                col_sb = small.tile([128, PCB], f32, tag="colsb")
                nc.vector.tensor_copy(col_sb[:], psum_col[:, csl])
                nc.gpsimd.dma_start(col_in[panel][:], col_sb[:])
                if single:
                    nc.gpsimd.dma_start(col_out[panel][:], col_in[panel][:])
                else:
                    nc.gpsimd.collective_compute(
                        "AllReduce", mybir.AluOpType.add, replica_groups=groups,
                        ins=[col_in[panel].opt()], outs=[col_out[panel].opt()])
                colsb2 = small.tile([128, PCB], f32, tag="cs2")
                nc.gpsimd.dma_start(colsb2[:], col_out[panel][:])
                sqc = small.tile([128, PCB], f32, tag="sqc")
                nc.scalar.sqrt(sqc[:], colsb2[:])
                nc.vector.reciprocal(s_c[:, csl], sqc[:])
                for j in range(PCB):
                    cb = panel * PCB + j
                    nc.vector.tensor_scalar(
                        xc[:, panel % 2, j], ei[:, cb], s_c[:, cb:cb + 1],
                        None, ALU.mult)

            # drain Q for the final panel
            emit_q(PAN - 1)

            # ---------------- phase B
            # s_r and Xr
            rowsum = small.tile([128, UB], f32, tag="sqr")
            for ub in range(UB):
                nc.vector.tensor_reduce(rowsum[:, ub:ub + 1],
                                        rowsum_parts[:, ub],
                                        mybir.AxisListType.X, ALU.add)
            sqr = small.tile([128, UB], f32, tag="sqr2")
            nc.scalar.sqrt(sqr[:], rowsum[:])
            nc.vector.reciprocal(s_r[:], sqr[:])
            for ub in range(UB):
                nc.scalar.activation(xr[:, ub], xr[:, ub], AF.Copy,
                                     scale=s_r[:, ub:ub + 1])

            # P: stationary Xr[ub], moving natural cache; out P^T blocks.
            p_in = dram.tile([CB, 128, 128], bf16)
            p_out = dram.tile([UB, 128, 128], bf16)
            uu_view = upd_user.rearrange("(ub p) d -> p ub d", p=128)
            ui_view = upd_item.rearrange("(ub p) d -> p ub d", p=128)

            def emit_p_sub(sub, eng):
                pp = ps2k.tile([128, 512], f32, tag="s2k")
                for ub in range(UB):
                    nc.tensor.matmul(
                        pp[:], xr[:, ub], cache[:, ub, sub * 512:(sub + 1) * 512],
                        start=(ub == 0), stop=(ub == UB - 1),
                        skip_group_check=True)
                pst = pstp.tile([128, 512], bf16, tag="pst")
                if eng is None:
                    nc.scalar.activation(pst[:], pp[:], AF.Copy)
                else:
                    eng.tensor_copy(pst[:], pp[:])
                blk = slice(sub * 4, (sub + 1) * 4)
                nc.sync.dma_start(p_in[blk].rearrange("b d c -> d b c"), pst[:])

            def emit_item_finish(ub):
                """out_item[ub] = LReLU(s_r*(q0 + (q1*uown)@W2) + iown)"""
                g = small.tile([128, D], f16, tag="g")
                nc.vector.tensor_mul(g[:], psum_q[:, ub, D:2 * D], uown[:, ub])
                ptg = psum_small.tile([D, 128], f16, tag="pe")
                nc.tensor.transpose(ptg[:], g[:], ident[:])
                gt = small.tile([D, 128], f16, tag="ibt")
                nc.scalar.activation(gt[:], ptg[:], AF.Copy)
                pg = psum_small.tile([128, D], f32, tag="pe")
                nc.tensor.matmul(pg[:], gt[:], w2_hf[:], start=True, stop=True)
                ta = small.tile([128, D], f32, tag="ft")
                nc.scalar.activation(ta[:], pg[:], AF.Copy)
                tb = small.tile([128, D], f32, tag="ft")
                nc.vector.scalar_tensor_tensor(
                    tb[:], psum_q[:, ub, 0:D], 1.0, ta[:], ALU.mult, ALU.add)
                tcm = small.tile([128, D], f32, tag="ft")
                nc.vector.scalar_tensor_tensor(
                    tcm[:], tb[:], s_r[:, ub:ub + 1], iown[:, ub],
                    ALU.mult, ALU.add)
                nc.vector.scalar_tensor_tensor(
                    out_stage[:, ub], tcm[:], 0.2, tcm[:], ALU.mult, ALU.max)

            for sub in range(16):
                emit_p_sub(sub, nc.vector if sub % 2 else None)
                if sub % 2 == 0:
                    emit_item_finish(sub // 2)

            nc.scalar.dma_start(ui_view[:], out_stage[:])
            if single:
                nc.sync.dma_start(p_out[:], p_in[0:UB])
            else:
                nc.gpsimd.collective_compute(
                    "ReduceScatter", mybir.AluOpType.add, replica_groups=groups,
                    ins=[p_in.opt()], outs=[p_out.opt()])

            # ---------------- finish user side (post-RS)
            pid = nc.vector.partition_id()
            rspv = p_out[:].rearrange("(h b) d c -> d h b c", h=2)
            for h in range(2):
                rh = pstp.tile([128, UB // 2, 128], bf16, tag="pst",
                               name=f"rsph{h}")
                nc.gpsimd.dma_start(rh[:], rspv[:, h])
                for k in range(UB // 2):
                    ub = h * (UB // 2) + k
                    rtt = psum_small.tile([128, 128], bf16, tag="pe")
                    nc.tensor.transpose(rtt[:], rh[:, k], identbf[:])
                    g2 = small.tile([128, D], f16, tag="g")
                    nc.vector.tensor_mul(g2[:], rtt[:, D:2 * D], iown[:, ub])
                    ptg2 = psum_small.tile([D, 128], f16, tag="pe")
                    nc.tensor.transpose(ptg2[:], g2[:], ident[:])
                    gt2 = small.tile([D, 128], f16, tag="ibt")
                    nc.scalar.activation(gt2[:], ptg2[:], AF.Copy)
                    pg2 = psum_small.tile([128, D], f32, tag="pe")
                    nc.tensor.matmul(pg2[:], gt2[:], w2_hf[:], start=True,
                                     stop=True)
                    if single:
                        sc_ap = s_c[:, ub:ub + 1]
                    else:
                        sc_ap = s_c[:, ds(pid * UB + ub, 1)]
                    t1 = small.tile([128, D], f32, tag="ft")
                    nc.vector.scalar_tensor_tensor(
                        t1[:], rtt[:, 0:D], sc_ap, uown[:, ub],
                        ALU.mult, ALU.add)
                    s2u = small.tile([128, D], f32, tag="ft")
                    nc.vector.scalar_tensor_tensor(
                        s2u[:], pg2[:], sc_ap, t1[:], ALU.mult, ALU.add)
                    nc.vector.scalar_tensor_tensor(
                        out_stage[:, ub], s2u[:], 0.2, s2u[:],
                        ALU.mult, ALU.max)
            nc.scalar.dma_start(uu_view[:], out_stage[:])

    nc.compile()
    return nc


def _get_nc(dbg=False):
    key = ("nc", dbg)
    if key not in _CACHE:
        _CACHE[key] = _build(dbg)
    return _CACHE[key]


def make_in_maps(user_embeddings, item_embeddings, adjacency_matrix, W1, W2):
    adj = np.ascontiguousarray(np.asarray(adjacency_matrix, dtype=np.float32))
    ue = np.ascontiguousarray(np.asarray(user_embeddings, dtype=np.float32))
    ie = np.ascontiguousarray(np.asarray(item_embeddings, dtype=np.float32))
    w1 = np.ascontiguousarray(np.asarray(W1, dtype=np.float32))
    w2 = np.ascontiguousarray(np.asarray(W2, dtype=np.float32))
    in_maps = []
    for k in range(NCORES):
        sl = slice(k * U, (k + 1) * U)
        in_maps.append({
            "adj": np.ascontiguousarray(adj[sl]),
            "user_full": ue,
            "item_full": ie,
            "user_own": np.ascontiguousarray(ue[sl]),
            "item_own": np.ascontiguousarray(ie[sl]),
            "w1": w1,
            "w2": w2,
        })
    return in_maps


def assemble(results):
    upd_user = np.concatenate([results[k]["upd_user"] for k in range(NCORES)], 0)
    upd_item = np.concatenate([results[k]["upd_item"] for k in range(NCORES)], 0)
    return upd_user, upd_item


def kernel(user_embeddings, item_embeddings, adjacency_matrix, W1, W2):
    import time
    import concourse.bass_utils as bass_utils
    nc = _get_nc()
    in_maps = make_in_maps(user_embeddings, item_embeddings, adjacency_matrix,
                           W1, W2)
    last = None
    for attempt in range(3):
        try:
            res = bass_utils.run_bass_kernel_spmd(
                nc, in_maps, core_ids=list(range(NCORES)), trace=False)
            return assemble(res.results)
        except Exception as e:  # transient NRT/axon failures
            last = e
            time.sleep(10)
    raise last
